# revision 26
# baseline (speedup 1.0000x reference)
"""Multi-head self-attention (RMSNorm + causal MHA + out-proj) on 8 TRN2 cores.

Sharding (tensor-parallel hint): core c handles batch b = c//4 and head group
hg = c%4 (4 of 16 heads). Each core computes a PARTIAL output (its heads'
slice of the out-projection contraction); the host sums the 4 partials per
batch and transposes back.

Device kernel (per core), restructured for steady-state pipelining across
For_i reps — per-rep state lives in name-keyed rings with bufs=2 so engine
streams drift across the iteration boundary instead of serializing on
single-buffered SBUF:

  - x is consumed RAW by all projections; RMSNorm's rstd is folded into the
    PSUM evacuations instead of materializing xn (saves a full [P,KT,T] DVE
    pass and removes rstd from the projection critical path). Q/K evacs
    multiply by the feature-broadcast rstd row (per token column); V evacs
    multiply by a token-major rstdT column (per token partition).
  - sum-of-squares accumulates on DVE in bf16 (square+add chain over the 8
    feature tiles), then 4x 512-col ones-matmuls give the partition-reduced,
    partition-broadcast ms; 16x 1-col matmuls (xqs tile as stationary, ones
    column moving) give the token-major msT. ACT sqrt + DVE reciprocal.
  - bf16 compute on TensorE; norm weight folded into projection weights on
    the host; weights shipped pre-transposed bf16.
  - Q stored with a head PAIR stacked on partition halves; K zero-padded to
    128 contraction rows per head (64-row matmuls run ~1.75x slower per
    column on this HW), so every score matmul is full-width and the paired
    Q moving data is annihilated exactly by the zero weight rows.
  - Causal: score matmuls and exp cover only the causal region; diagonal
    tiles packed contiguously in PSUM; in-block masks via affine_select on
    the otherwise-idle Pool engine. Softmax denominator fused into PV via a
    trailing ones column in V (l lands at out-partition 64, ctx at 0:64);
    ctx normalized straight out of PSUM (DVE reciprocal of the l row, Pool
    partition_broadcast of 1/l, DVE multiply).
  - The attention stream is rate-matched PE vs ACT(exp); ALL other PE work
    (next chunk's QKV projections, previous chunk's out-projection) is
    pumped as filler units between attention groups so the PE never idles
    waiting on the exp pipeline, and the out-projection DMAs stream out
    per (e, chunk) slice.
"""

import os
from contextlib import ExitStack

import numpy as np
import ml_dtypes

import concourse.bass as bass
import concourse.tile as tile
from concourse import bacc, mybir
from concourse.bass_utils import run_bass_kernel_spmd

F32 = mybir.dt.float32
BF16 = mybir.dt.bfloat16
AF = mybir.ActivationFunctionType
P = 128
DD = 64
T = 2048
D = 1024
NH = 4            # heads per core
KT = D // P       # 8 feature tiles
TT = T // P       # 16 token tiles
TC = T // 512     # 4 query chunks
N_CORES = 8
EPS = 1e-6


def build_kernel(nc, reps=1):
    xT_d = nc.dram_tensor("xT", [P, KT * T], BF16, kind="ExternalInput")
    wqk_d = nc.dram_tensor("wqkT", [P, KT * 512], BF16, kind="ExternalInput")
    wv_d = nc.dram_tensor("wvT", [P, KT * 256], BF16, kind="ExternalInput")
    wo_d = nc.dram_tensor("woT", [P, 2 * D], BF16, kind="ExternalInput")
    outT_d = nc.dram_tensor("outT", [P, 8 * T], BF16, kind="ExternalOutput")

    phase = os.environ.get("KERNEL_PHASE", "full")

    with tile.TileContext(nc) as tc, ExitStack() as ctx:
        consts = ctx.enter_context(tc.tile_pool(name="consts", bufs=1))
        persist = ctx.enter_context(tc.tile_pool(name="persist", bufs=1))
        dbl = ctx.enter_context(tc.tile_pool(name="dbl", bufs=2))
        sgl = ctx.enter_context(tc.tile_pool(name="sgl", bufs=1))
        obp = ctx.enter_context(tc.tile_pool(name="obp", bufs=3))
        epool = ctx.enter_context(tc.tile_pool(name="epool", bufs=3))
        rlp = ctx.enter_context(tc.tile_pool(name="rlp", bufs=4))
        sqp = ctx.enter_context(tc.tile_pool(name="sqp", bufs=2))
        sps = ctx.enter_context(tc.tile_pool(name="sps", bufs=2, space="PSUM"))
        ctxp = ctx.enter_context(tc.tile_pool(name="ctxp", bufs=2, space="PSUM"))
        mmp = ctx.enter_context(tc.tile_pool(name="mmp", bufs=2, space="PSUM"))

        # ---- loop-invariant prelude ------------------------------------
        ones_bf = consts.tile([P, P], BF16)
        nc.vector.memset(ones_bf[:], 1.0)
        eps_sb = consts.tile([P, 1], F32)
        nc.vector.memset(eps_sb[:], EPS)

        wqk_bf = persist.tile([P, KT, 512], BF16)
        wv_bf = persist.tile([P, KT, 256], BF16)
        wo_bf = persist.tile([P, 2, D], BF16)
        nc.sync.dma_start(wqk_bf[:], wqk_d.ap())
        nc.sync.dma_start(wv_bf[:], wv_d.ap())
        nc.sync.dma_start(wo_bf[:], wo_d.ap())

        # K zero-padded to 128 contraction rows per head (64-row matmuls run
        # at ~1.75x the per-column cost of 128-row ones on HW): head h's K
        # occupies rows 64*(h%2)..+64, the other 64 rows are zero so the
        # paired Q moving data (both heads stacked) is annihilated exactly.
        # V with a leading ones column (PV emits l at out-partition 0, ctx at
        # 1:65). Both ping-pong across reps; the pad/ones regions are never
        # rewritten, so they are initialized once here.
        VW = 66  # 65 used (1 ones + 64 d), padded for alignment
        kzs = [persist.tile([P, NH, T], BF16, name=f"kz{i}") for i in range(2)]
        vss = [persist.tile([P, NH, TT, VW], BF16, name=f"vs{i}") for i in range(2)]
        for kz in kzs:
            for h in range(NH):
                z = slice(64, 128) if h % 2 == 0 else slice(0, 64)
                nc.vector.memset(kz[z, h, :], 0.0)
        for vs in vss:
            nc.gpsimd.memset(vs[:, :, :, 64:65], 1.0)

        def emit_body(kz, vs, iv=None):
            # ---- per-rep double-buffered state -------------------------
            xbf = dbl.tile([P, KT, T], BF16, name="xbf")
            xqs = sgl.tile([P, T], BF16, name="xqs")
            scr = sgl.tile([P, T // 2], BF16, name="scr")
            rstd = dbl.tile([P, T], F32, name="rstd")
            rstdT = dbl.tile([P, 16], F32, name="rstdT")
            QTd = dbl.tile([P, 2, T], BF16, name="QTd")
            ctxn = dbl.tile([P, 2, T], BF16, name="ctxn")

            # ---- x load + sum-of-squares (DVE) -------------------------
            for kt in range(KT):
                nc.sync.dma_start(xbf[:, kt, :], xT_d.ap()[:, kt * T : (kt + 1) * T])
            nc.vector.tensor_mul(xqs[:], xbf[:, 0, :], xbf[:, 0, :])
            for kt in range(1, KT):
                for hf in range(2):
                    hs = slice(1024 * hf, 1024 * (hf + 1))
                    nc.vector.tensor_mul(scr[:, :], xbf[:, kt, hs], xbf[:, kt, hs])
                    nc.vector.tensor_add(xqs[:, hs], xqs[:, hs], scr[:, :])

            # ---- rstd (feature-broadcast) + rstdT (token-major) --------
            for c in range(TC):
                cs = slice(512 * c, 512 * (c + 1))
                msps = mmp.tile([P, 512], F32, tag="mm", name="msps")
                nc.tensor.matmul(msps[:], ones_bf[:], xqs[:, cs], start=True, stop=True)
                sqf = sqp.tile([P, 512], F32, name="sqf")
                nc.scalar.activation(
                    sqf[:], msps[:], AF.Sqrt, bias=eps_sb[:, 0:1], scale=1.0 / D
                )
                nc.vector.reciprocal(rstd[:, cs], sqf[:])
            msT = mmp.tile([P, 512], F32, tag="mm", name="msT")
            for tt in range(TT):
                nc.tensor.matmul(
                    msT[:, tt : tt + 1],
                    xqs[:, P * tt : P * (tt + 1)],
                    ones_bf[:, 0:1],
                    start=True, stop=True,
                )
            sqT = sqp.tile([P, 16], F32, name="sqT")
            nc.scalar.activation(
                sqT[:], msT[:, 0:16], AF.Sqrt, bias=eps_sb[:, 0:1], scale=1.0 / D
            )
            nc.vector.reciprocal(rstdT[:], sqT[:])

            # ---- projection units (pumped as attention fillers) --------
            def qk_unit(pair, is_k, c):
                ft = 2 * is_k + pair
                cs = slice(512 * c, 512 * (c + 1))
                qkps = mmp.tile([P, 512], F32, tag="mm", name="qkps")
                for kt in range(KT):
                    nc.tensor.matmul(
                        qkps[:],
                        wqk_bf[:, kt, P * ft : P * (ft + 1)],
                        xbf[:, kt, cs],
                        start=(kt == 0), stop=(kt == KT - 1),
                    )
                with nc.allow_low_precision(reason="qk feeds bf16 matmuls"):
                    if is_k:
                        nc.vector.tensor_mul(
                            kz[0:64, 2 * pair, cs], qkps[0:64, :], rstd[0:64, cs]
                        )
                        nc.vector.tensor_mul(
                            kz[64:128, 2 * pair + 1, cs],
                            qkps[64:128, :], rstd[64:128, cs],
                        )
                    else:
                        nc.vector.tensor_mul(QTd[:, pair, cs], qkps[:], rstd[:, cs])

            def v_unit(tt):
                vps = mmp.tile([P, 512], F32, tag="mm", name="vps")
                for kt in range(KT):
                    nc.tensor.matmul(
                        vps[:, :256],
                        xbf[:, kt, P * tt : P * (tt + 1)],
                        wv_bf[:, kt, :],
                        start=(kt == 0), stop=(kt == KT - 1),
                    )
                vh = vps[:, 0:256].rearrange("p (h d) -> p h d", h=NH)
                with nc.allow_low_precision(reason="v feeds bf16 matmuls"):
                    nc.vector.tensor_scalar_mul(
                        vs[:, :, tt, 0:64], vh[:, :, :], rstdT[:, tt : tt + 1]
                    )

            def out_unit(c, e):
                cs = slice(512 * c, 512 * (c + 1))
                ops = mmp.tile([P, 512], F32, tag="mm", name="ops")
                for ct in range(2):
                    nc.tensor.matmul(
                        ops[:],
                        wo_bf[:, ct, P * e : P * (e + 1)],
                        ctxn[:, ct, cs],
                        start=(ct == 0), stop=(ct == 1),
                    )
                osb = obp.tile([P, 512], BF16, name="osb")
                nc.scalar.copy(osb[:], ops[:])
                nc.sync.dma_start(
                    outT_d.ap()[:, T * e + 512 * c : T * e + 512 * (c + 1)],
                    osb[:],
                )

            # ---- filler pump -------------------------------------------
            fillers = []

            def pump(n=1):
                for _ in range(n):
                    if not fillers:
                        return
                    fillers.pop(0)()

            def drain():
                while fillers:
                    fillers.pop(0)()

            # ---- attention ---------------------------------------------
            post = []

            def drain_post(keep=0):
                while len(post) > keep:
                    post.pop(0)()

            def emit_attn_chunk(h, c):
                pair = h // 2
                Q = QTd[:, pair, :]
                K_ = kz[:, h, :]
                ctx_ps = ctxp.tile([P, 512], F32, name="ctx_ps")
                q0 = 512 * c

                def pv(e_t, j, coff, ccols, ecols, stop):
                    nc.tensor.matmul(
                        ctx_ps[0:65, coff : coff + ccols],
                        vs[:, h, j, 0:65],
                        e_t[:, ecols[0] : ecols[1]],
                        start=(j == 0), stop=stop,
                    )

                for g in range(2 * c):
                    j0, j1 = 2 * g, 2 * g + 1
                    sst = sps.tile([P, 1024], F32, name="sst")
                    nc.tensor.matmul(
                        sst[:, 0:512], K_[:, P * j0 : P * (j0 + 1)],
                        Q[:, q0 : q0 + 512], start=True, stop=True,
                    )
                    nc.tensor.matmul(
                        sst[:, 512:1024], K_[:, P * j1 : P * (j1 + 1)],
                        Q[:, q0 : q0 + 512], start=True, stop=True,
                    )
                    drain_post()
                    expS = epool.tile([P, 1024], BF16, name="expS")
                    nc.scalar.activation(expS[:], sst[:], AF.Exp, scale=0.125)
                    post.append(lambda e_t=expS, a=j0, b=j1: (
                        pv(e_t, a, 0, 512, (0, 512), False),
                        pv(e_t, b, 0, 512, (512, 1024), False),
                    ))
                    pump()

                # diagonal group A: tiles 4c (W=512 @ col 0), 4c+1 (W=384 @ col 512)
                j0, j1 = 4 * c, 4 * c + 1
                sst = sps.tile([P, 1024], F32, name="sst")
                nc.tensor.matmul(
                    sst[:, 0:512], K_[:, P * j0 : P * (j0 + 1)],
                    Q[:, q0 : q0 + 512], start=True, stop=True,
                )
                nc.tensor.matmul(
                    sst[:, 512:896], K_[:, P * j1 : P * (j1 + 1)],
                    Q[:, q0 + 128 : q0 + 512], start=True, stop=True,
                )
                drain_post()
                expS = epool.tile([P, 1024], BF16, name="expS")
                nc.scalar.activation(expS[:, 0:896], sst[:, 0:896], AF.Exp, scale=0.125)
                mA = expS[:, 0:1024].rearrange("p (a b d) -> p a b d", a=2, d=P)[
                    :, :, 0, :
                ]
                nc.gpsimd.affine_select(
                    out=mA, in_=mA,
                    compare_op=mybir.AluOpType.is_ge, fill=0.0, base=0,
                    pattern=[[0, 2], [1, P]], channel_multiplier=-1,
                )
                post.append(lambda e_t=expS, a=j0, b=j1: (
                    pv(e_t, a, 0, 512, (0, 512), False),
                    pv(e_t, b, 128, 384, (512, 896), False),
                ))
                pump()

                # diagonal group B: tiles 4c+2 (W=256 @ col 0), 4c+3 (W=128 @ col 256)
                j2, j3 = 4 * c + 2, 4 * c + 3
                sst = sps.tile([P, 1024], F32, name="sst")
                nc.tensor.matmul(
                    sst[:, 0:256], K_[:, P * j2 : P * (j2 + 1)],
                    Q[:, q0 + 256 : q0 + 512], start=True, stop=True,
                )
                nc.tensor.matmul(
                    sst[:, 256:384], K_[:, P * j3 : P * (j3 + 1)],
                    Q[:, q0 + 384 : q0 + 512], start=True, stop=True,
                )
                drain_post()
                expS = epool.tile([P, 1024], BF16, name="expS")
                nc.scalar.activation(expS[:, 0:384], sst[:, 0:384], AF.Exp, scale=0.125)
                mB = expS[:, 0:512].rearrange("p (a b d) -> p a b d", a=2, d=P)[
                    :, :, 0, :
                ]
                nc.gpsimd.affine_select(
                    out=mB, in_=mB,
                    compare_op=mybir.AluOpType.is_ge, fill=0.0, base=0,
                    pattern=[[0, 2], [1, P]], channel_multiplier=-1,
                )
                post.append(lambda e_t=expS, a=j2, b=j3: (
                    pv(e_t, a, 256, 256, (0, 256), False),
                    pv(e_t, b, 384, 128, (256, 384), True),
                ))
                post.append(lambda: emit_norm(h, c, ctx_ps))
                pump()

            def emit_norm(h, c, ctx_ps):
                # PV leaves ctx at out-partitions 0:64 and l = sum(exp) at
                # partition 64 (trailing ones column in V). Reciprocal the l
                # row into partition 0, Pool partition_broadcast 1/l to 64
                # rows, multiply into ctxn.
                pair, half = h // 2, h % 2
                cs = slice(512 * c, 512 * (c + 1))
                crows = slice(64 * half, 64 * half + 64)
                rl = rlp.tile([P, 512], F32, name="rl")
                nc.vector.reciprocal(rl[0:1, :], ctx_ps[64:65, :])
                nc.gpsimd.partition_broadcast(rl[0:64, :], rl[0:1, :])
                with nc.allow_low_precision(reason="ctx feeds bf16 matmul"):
                    nc.vector.tensor_mul(
                        ctxn[crows, pair, cs], ctx_ps[0:64, :], rl[0:64, :]
                    )

            # ---- schedule ----------------------------------------------
            if phase != "ldnorm":
                for pair in range(2):
                    for is_k in (0, 1):
                        qk_unit(pair, is_k, 0)
                for tt in range(4):
                    v_unit(tt)
            if phase in ("attn", "full"):
                for c in range(TC):
                    proj_f, out_f = [], []
                    if c < TC - 1:
                        for pair in range(2):
                            for is_k in (0, 1):
                                proj_f.append(
                                    lambda p=pair, k=is_k, cc=c + 1: qk_unit(p, k, cc)
                                )
                        for tt in range(4 * (c + 1), 4 * (c + 2)):
                            proj_f.append(lambda t=tt: v_unit(t))
                    if phase == "full" and c >= 2:
                        for e in range(D // P):
                            out_f.append(lambda cc=c - 2, ee=e: out_unit(cc, ee))
                    # round-robin so mm-ring neighbors evacuate on
                    # different engines (qk/v -> DVE, out -> ACT)
                    while proj_f or out_f:
                        if proj_f:
                            fillers.append(proj_f.pop(0))
                        if out_f:
                            fillers.append(out_f.pop(0))
                    for h in (1, 3, 0, 2):
                        emit_attn_chunk(h, c)
                drain_post()
                drain()
                if phase == "full":
                    for c in (2, 3):
                        for e in range(D // P):
                            out_unit(c, e)
            else:
                if phase == "proj":
                    for c in range(1, TC):
                        for pair in range(2):
                            for is_k in (0, 1):
                                qk_unit(pair, is_k, c)
                        for tt in range(4 * c, 4 * (c + 1)):
                            v_unit(tt)
            if phase != "full":
                dummy = dbl.tile([P, 1024], BF16, name="dummy")
                nc.vector.tensor_copy(dummy[:], xqs[:, 0:1024])
                nc.sync.dma_start(outT_d.ap()[:, 0:1024], dummy[:])

        if reps == 1:
            emit_body(kzs[0], vss[0])
        else:
            assert reps % 2 == 0, "reps must be even (2-deep ring pipelining)"
            with tc.For_i(0, reps // 2, 1) as iv:
                emit_body(kzs[0], vss[0], iv)
                emit_body(kzs[1], vss[1], iv)


_NC_CACHE = None


def _get_nc():
    global _NC_CACHE
    if _NC_CACHE is None:
        nc = bacc.Bacc(
            "TRN2", target_bir_lowering=False, debug=False, num_devices=N_CORES
        )
        build_kernel(nc)
        nc.compile()
        _NC_CACHE = nc
    return _NC_CACHE


def _tile_rows(a):
    """[R*128, C] -> [128, R*C] flat feature-tiled layout."""
    r, c = a.shape
    return np.ascontiguousarray(
        a.reshape(r // P, P, c).transpose(1, 0, 2).reshape(P, (r // P) * c)
    )


def make_in_maps(x, norm_weight, qkv_w, out_w):
    x = np.asarray(x, dtype=np.float32)
    norm_weight = np.asarray(norm_weight, dtype=np.float32)
    qkv_w = np.asarray(qkv_w, dtype=np.float32)
    out_w = np.asarray(out_w, dtype=np.float32)
    qkv_eff = qkv_w * norm_weight[None, :]
    bf = ml_dtypes.bfloat16
    in_maps = []
    for core in range(N_CORES):
        b, hg = core // 4, core % 4
        r0 = 256 * hg
        xT = _tile_rows(np.ascontiguousarray(x[b].T)).astype(bf)
        # wqk col blocks: [q-pair0, q-pair1, k-pair0, k-pair1] (128 each)
        wq = qkv_eff[r0 : r0 + 256]
        wk = qkv_eff[D + r0 : D + r0 + 256]
        wqk = np.concatenate([wq[:128], wq[128:], wk[:128], wk[128:]], 0).T
        wqkT = _tile_rows(np.ascontiguousarray(wqk)).astype(bf)
        wvT = _tile_rows(
            np.ascontiguousarray(qkv_eff[2 * D + r0 : 2 * D + r0 + 256].T)
        ).astype(bf)
        woT = _tile_rows(np.ascontiguousarray(out_w[:, r0 : r0 + 256].T)).astype(bf)
        in_maps.append({"xT": xT, "wqkT": wqkT, "wvT": wvT, "woT": woT})
    return in_maps


def gather_output(results):
    out = np.empty((2, T, D), np.float32)
    for b in range(2):
        acc = results[4 * b]["outT"].astype(np.float32).copy()
        for hg in range(1, 4):
            acc += results[4 * b + hg]["outT"]
        # [128, 8*T] -> [D, T] -> [T, D]
        out[b] = acc.reshape(P, D // P, T).transpose(1, 0, 2).reshape(D, T).T
    return out


def kernel(x, norm_weight, qkv_w, out_w):
    nc = _get_nc()
    in_maps = make_in_maps(x, norm_weight, qkv_w, out_w)
    res = run_bass_kernel_spmd(nc, in_maps, core_ids=list(range(N_CORES)))
    return gather_output(res.results)
